# revision 38
# baseline (speedup 1.0000x reference)
"""Block-sparse attention (local + vertical-strided causal mask) on 8 TRN2 cores.

Sharding: one head per NeuronCore (H=8, n_cores=8).

Per-core device algorithm (head h, residue r = 7-h):
  The 4096x4096 score matrix is processed at 128x128 granularity:
  "pair" i = q block-rows (2i, 2i+1) (128 q tokens), "chunk" = 128 k tokens
  (2 mask blocks of 64). Local window -> chunks c in [i-8, i] of K itself;
  vertical-strided blocks -> host-gathered K_vert (6 blocks of 64, kb = 8j+r),
  processed as 3 chunks shared by all cores.

  S^T orientation: S^T[k,q] = kT_chunk.T @ qT_pair  (PE, bf16; sm_scale
  pre-folded into qT on host)
  window-start / vert-validity masks: rank-2 additive -C matmuls into the
  same PSUM region (PE); exp underflows those entries to exact 0
  P^T = exp(S^T)                                     (ACT, one call per group)
  diag triangle: multiplicative bf16 mask            (DVE)
  oacc[q, 0:129] += P^T_chunk.T @ [V | 1]_chunk      (PE, PSUM-accumulated)
  col 128 of oacc = softmax denominator; copied PSUM->SBUF (DVE) and stored
  unnormalized; the host divides by the denominator column.
"""

import numpy as np
import ml_dtypes

BF16 = ml_dtypes.bfloat16

H = 8
S = 4096
D = 128
BLK = 64
NB = S // BLK        # 64 block rows
NPAIR = NB // 2      # 32 row pairs
NVSLOT = 6           # usable vertical slots (kb = 8j + r <= 47)
NVC = NVSLOT // 2    # 3 vertical chunks
GROUP = 8            # PSUM staging slots per exp group (8 * 128 f32 = 2 banks)

NEGC = 28672.0       # additive mask constant; bf16-exact, exp() underflows to 0


def make_schedule():
    """Global ordered visit list. visit = (kind, idx, pair)
    kind "local": idx = chunk c (k blocks 2c, 2c+1)
    kind "vert":  idx = vc (K_vert slots 2vc, 2vc+1; pair i gets vc iff
    8*vc+8 <= i, i.e. the vert chunk lies fully before the local window)

    Wavefront order: all visits of pair i together (vert, then locals old
    to new, diag last) — input demand is then sequential in q and grows by
    one k-chunk per pair, so DMA staging never starves the pipeline."""
    visits = []
    for i in range(NPAIR):
        for vc in range(NVC):
            if 8 * vc + 8 <= i:
                visits.append(("vert", vc, i))
        for c in range(max(0, i - 8), i + 1):
            visits.append(("local", c, i))
    return visits


def mask_visit_order():
    """Visits that need an additive rank-2 mask, in schedule order.
    vert: per-(vc, i) validity — pruned when every head's slots are valid
    (kb_max = 16*vc+8+r <= 2i-16 for all r < 8, i.e. i >= 8*vc+16);
    local with idx == i-8: window-start mask (always needed)."""
    out = []
    for kind, idx, i in make_schedule():
        if kind == "vert" and i < 8 * idx + 16:
            out.append((kind, idx, i))
        elif kind == "local" and idx == i - 8:
            out.append((kind, idx, i))
    return out


NMASK = len(mask_visit_order())

_PROGRAMS = {}


def _build_program(sm_scale, pv_delay=4, group=GROUP, stage_bufs=2, pt_bufs=None,
                   ob_bufs=2):
    if pt_bufs is None:
        pt_bufs = pv_delay + 2
    import concourse.bass as bass
    import concourse.mybir as mybir
    import concourse.tile as tile
    from concourse import bacc

    fp32 = mybir.dt.float32
    bf16 = mybir.dt.bfloat16

    nc = bacc.Bacc("TRN2", target_bir_lowering=False, debug=False, num_devices=H)

    qt_d = nc.dram_tensor("qt", [D, S], bf16, kind="ExternalInput").ap()
    kt_d = nc.dram_tensor("kt", [D, S], bf16, kind="ExternalInput").ap()
    ktv_d = nc.dram_tensor("ktv", [D, NVSLOT * BLK], bf16, kind="ExternalInput").ap()
    vvaug_d = nc.dram_tensor("vvaug", [128, NVC * (D + 1)], bf16,
                             kind="ExternalInput").ap()
    vaug_d = nc.dram_tensor("vaug", [128, NPAIR, D + 1], bf16, kind="ExternalInput").ap()
    # vmu = per-mask-visit [128, 128] lhsT slices (rows 0/1 = invalid
    # indicators, rest zero) + wpat last; padded to K=128 so the mask
    # matmuls use the standard PE tile config (K=2 configs run ~2.7x slower)
    vmu_d = nc.dram_tensor("vmu", [128, (NMASK + 1) * 128], bf16,
                           kind="ExternalInput").ap()
    tri_d = nc.dram_tensor("tri", [128, 128], bf16, kind="ExternalInput").ap()
    o_d = nc.dram_tensor("o", [128, NPAIR, D + 1], fp32, kind="ExternalOutput").ap()

    visits = make_schedule()
    first = {}
    last = {}
    for g, (kind, idx, i) in enumerate(visits):
        first.setdefault(i, g)
        last[i] = g
    # PSUM start_tensor_calc zeroes the full 2KB bank (zero-region), so only
    # the first matmul touching an oacc tile may carry start=True.
    tile_first = {}
    for g, (kind, idx, i) in enumerate(visits):
        tile_first.setdefault(i // 3, g)
    mask_idx = {v: mi for mi, v in enumerate(mask_visit_order())}

    with tile.TileContext(nc) as tc:
        with (
            tc.tile_pool(name="big", bufs=1) as big,
            tc.tile_pool(name="stage", bufs=stage_bufs, space="PSUM") as stagep,
            tc.tile_pool(name="oacc", bufs=4, space="PSUM") as oaccp,
            tc.tile_pool(name="pt", bufs=pt_bufs) as ptp,
            tc.tile_pool(name="ob", bufs=ob_bufs) as obp,
        ):
            _emit_body(nc, tc, locals(), sm_scale, pv_delay=pv_delay, group=group)
    nc.compile()
    return nc


def _emit_body(nc, tc, env, sm_scale, pv_delay=4, group=GROUP):
    GROUP = group
    import concourse.mybir as mybir

    fp32 = mybir.dt.float32
    bf16 = mybir.dt.bfloat16
    big, stagep, oaccp, ptp, obp = (
        env["big"], env["stagep"], env["oaccp"], env["ptp"], env["obp"]
    )
    qt_d, kt_d, ktv_d, vvaug_d, vaug_d, vmu_d, tri_d, o_d = (
        env["qt_d"], env["kt_d"], env["ktv_d"], env["vvaug_d"], env["vaug_d"],
        env["vmu_d"], env["tri_d"], env["o_d"],
    )
    visits, first, last, tile_first, mask_idx = (
        env["visits"], env["first"], env["last"], env["tile_first"],
        env["mask_idx"],
    )
    n_groups = (len(visits) + GROUP - 1) // GROUP

    qt = big.tile([D, S], bf16)
    kt = big.tile([D, S], bf16)
    ktv = big.tile([D, NVSLOT * BLK], bf16)
    vvaug = big.tile([128, NVC * (D + 1)], bf16)
    vaug = big.tile([128, NPAIR, D + 1], bf16)
    vmu = big.tile([128, (NMASK + 1) * 128], bf16)
    tri = big.tile([128, 128], bf16)

    wpat = vmu[:, 0:128]

    # Inputs spread across the three DMA queues (SP-HW, ACT-HW, Pool-SW) so
    # the first groups' data lands fast; later chunks trail in use order.
    def kt_dma(a, b):
        nc.sync.dma_start(out=kt[:, a:b], in_=kt_d[:, a:b])

    def qt_dma(a, b):
        nc.scalar.dma_start(out=qt[:, a:b], in_=qt_d[:, a:b])

    def vaug_dma(a, b):
        nc.gpsimd.dma_start(out=vaug[:, a:b], in_=vaug_d[:, a:b])

    wtile = big.tile([128, 512], bf16)
    nc.vector.memset(wtile[:], 0.0)

    def vmu_dma(a, b, eng):
        eng.dma_start(out=vmu[:, a:b], in_=vmu_d[:, a:b])

    kt_dma(0, 512)
    kt_dma(512, 1024)
    nc.sync.dma_start(out=ktv[:], in_=ktv_d[:])
    kt_dma(1024, 2048)
    kt_dma(2048, 4096)
    vmu_dma(1152, 3200, nc.sync)
    vmu_dma(3200, (NMASK + 1) * 128, nc.sync)
    qt_dma(0, 256)
    qt_dma(256, 1024)
    nc.scalar.dma_start(out=vvaug[:], in_=vvaug_d[:])
    qt_dma(1024, 2048)
    qt_dma(2048, 4096)
    # vmu staged in mask-emission use order; wpat is slice 0
    nc.gpsimd.dma_start(out=tri[:], in_=tri_d[:])
    vmu_dma(0, 640, nc.gpsimd)
    vaug_dma(0, 6)
    vmu_dma(640, 1152, nc.gpsimd)
    vaug_dma(6, 18)
    vaug_dma(18, 32)

    # PE p-state warmup: stream throwaway matmuls on a memset tile so the
    # array is ramping while the first inputs arrive.
    warm = stagep.tile([128, GROUP * 128], fp32, tag="stage")
    for w in range(6):
        nc.tensor.matmul(
            warm[:, (w % 2) * 512 : (w % 2 + 1) * 512],
            wtile[:, 0:128],
            wtile[:, 0:512],
            start=True,
            stop=True,
            skip_group_check=True,
        )

    oacc_tiles = {}  # pair-group (i//3) -> psum tile [128, 3, 129]
    pending_pv = []  # software pipeline: PV of group gi-d emitted
    # after S^T of group gi so PE streams while ACT/DVE process gi-d

    for gi in range(n_groups):
        gvis = visits[gi * GROUP : (gi + 1) * GROUP]
        n = len(gvis)
        stage = stagep.tile([128, GROUP * 128], fp32, tag="stage")
        ptt = ptp.tile([128, GROUP * 128], bf16, tag="pt")

        # --- S^T matmuls, batched over runs of consecutive pairs
        # sharing one k-chunk, split at 4-slot (one PSUM bank) bounds.
        # start=True only on the first run per bank (bank zero-region).
        s = 0
        seen_banks = set()
        while s < n:
            kind, idx, i0 = gvis[s]
            e = s + 1
            while (
                e < n
                and e % 4 != 0
                and gvis[e][0] == kind
                and gvis[e][1] == idx
                and gvis[e][2] == gvis[e - 1][2] + 1
            ):
                e += 1
            ln = e - s
            lhsT = (
                kt[:, idx * 128 : (idx + 1) * 128]
                if kind == "local"
                else ktv[:, idx * 128 : (idx + 1) * 128]
            )
            bank = s // 4
            nc.tensor.matmul(
                stage[:, s * 128 : e * 128],
                lhsT,
                qt[:, i0 * 128 : (i0 + ln) * 128],
                start=bank not in seen_banks,
                stop=True,
                skip_group_check=True,
            )
            seen_banks.add(bank)
            # rank-2 additive masks (window-start / vert validity) for the
            # slots of this run, accumulated into the same PSUM region
            for s2 in range(s, e):
                mv = mask_idx.get(gvis[s2])
                if mv is not None:
                    nc.tensor.matmul(
                        stage[:, s2 * 128 : (s2 + 1) * 128],
                        vmu[:, (mv + 1) * 128 : (mv + 2) * 128],
                        wpat,
                        start=False,
                        stop=True,
                        skip_group_check=True,
                    )
            s = e

        if len(pending_pv) >= pv_delay:
            pending_pv.pop(0)()

        # --- exp for the group (sm_scale pre-folded into qt on host)
        nc.scalar.activation(
            out=ptt[:, 0 : n * 128],
            in_=stage[:, 0 : n * 128],
            func=mybir.ActivationFunctionType.Exp,
        )

        # --- diag triangle mask (DVE, multiplicative bf16)
        for s, (kind, idx, i) in enumerate(gvis):
            if kind == "local" and idx == i:
                sl = slice(s * 128, (s + 1) * 128)
                nc.vector.tensor_mul(ptt[:, sl], ptt[:, sl], tri[:])

        # --- PV matmuls + epilogue (deferred pv_delay groups)
        def make_pv(gi, gvis, ptt):
            def emit_pv():
                for s, (kind, idx, i) in enumerate(gvis):
                    g = gi * GROUP + s
                    pg = i // 3
                    if pg not in oacc_tiles:
                        oacc_tiles[pg] = oaccp.tile(
                            [128, 3, D + 1], fp32, tag="oacc", name=f"oacc{pg}"
                        )
                    oacc = oacc_tiles[pg]
                    if kind == "local":
                        rhs = vaug[:, idx]
                    else:
                        rhs = vvaug[:, idx * (D + 1) : (idx + 1) * (D + 1)]
                    nc.tensor.matmul(
                        oacc[:, i % 3],
                        ptt[:, s * 128 : (s + 1) * 128],
                        rhs,
                        start=(g == tile_first[i // 3]),
                        stop=(g == last[i]),
                        skip_group_check=True,
                    )
                    # epilogue once per oacc tile (after its last pair
                    # closes): one DVE read of the PSUM bank into SBUF,
                    # then an unnormalized store (host divides by col 128).
                    pg_pairs = [p for p in (3 * pg, 3 * pg + 1, 3 * pg + 2)
                                if p < NPAIR]
                    if i == pg_pairs[-1] and g == last[i]:
                        npp = len(pg_pairs)
                        osb = obp.tile([128, 3, D + 1], fp32, tag="osb")
                        nc.vector.tensor_copy(osb[:, 0:npp], oacc[:, 0:npp])
                        nc.sync.dma_start(
                            out=o_d[:, 3 * pg : 3 * pg + npp, :],
                            in_=osb[:, 0:npp],
                        )
            return emit_pv

        pending_pv.append(make_pv(gi, gvis, ptt))
    for f in pending_pv:
        f()


def _get_program(smv=0.08838834764831845):
    key = float(smv)
    if key not in _PROGRAMS:
        _PROGRAMS[key] = _build_program(key)
    return _PROGRAMS[key]


def _host_inputs(q, k, v, sm_scale):
    """Per-core input dicts (host-side shard + layout)."""
    q = np.asarray(q, dtype=np.float32)
    k = np.asarray(k, dtype=np.float32)
    v = np.asarray(v, dtype=np.float32)
    smv = float(np.asarray(sm_scale, dtype=np.float32))

    p = np.arange(128)
    tri = np.zeros((128, 128), dtype=BF16)
    tri[p[:, None] <= p[None, :]] = BF16(1.0)

    morder = mask_visit_order()
    ins = []
    for h in range(H):
        r = 7 - h
        qh, kh, vh = q[0, h], k[0, h], v[0, h]
        qt = np.ascontiguousarray((qh * smv).T).astype(BF16)
        kt = np.ascontiguousarray(kh.T).astype(BF16)
        vblocks = [8 * j + r for j in range(NVSLOT)]
        kv = np.concatenate([kh[b * BLK : (b + 1) * BLK] for b in vblocks], axis=0)
        ktv = np.ascontiguousarray(kv.T).astype(BF16)  # [128, 384]
        vaug = np.concatenate(
            [vh, np.ones((S, 1), np.float32)], axis=1
        ).astype(BF16)  # [4096, 129]
        vaug = np.ascontiguousarray(
            vaug.reshape(NPAIR, 128, D + 1).transpose(1, 0, 2)
        )  # [128, 32, 129]
        vv = np.concatenate([vh[b * BLK : (b + 1) * BLK] for b in vblocks], axis=0)
        vvaug = np.concatenate([vv, np.ones((NVSLOT * BLK, 1), np.float32)], axis=1)
        vvaug = np.ascontiguousarray(
            vvaug.astype(BF16).reshape(NVC, 128, D + 1).transpose(1, 0, 2)
        ).reshape(128, NVC * (D + 1))  # [128, 387]

        # vmu: per-mask-visit invalid indicators in rows 0 (cols<64) and
        # 1 (cols>=64), rows 2..127 zero; wpat appended last.
        vmu = np.zeros((128, (NMASK + 1) * 128), dtype=BF16)
        for mi, (kind, idx, i) in enumerate(morder):
            sl = slice((mi + 1) * 128, (mi + 2) * 128)
            if kind == "vert":
                slot = 2 * idx + (p >= 64).astype(np.int64)
                kb = 8 * slot + r
                u0 = (kb > 2 * i - 16).astype(np.float32)       # invalid for qb=2i
                u1 = (kb > 2 * i + 1 - 16).astype(np.float32)   # invalid for qb=2i+1
            else:  # window-start: valid iff (p >= 64 and col < 64)
                u0 = (p < 64).astype(np.float32)
                u1 = np.ones(128, np.float32)
            vmu[0, sl] = u0.astype(BF16)
            vmu[1, sl] = u1.astype(BF16)
        wsl = slice(0, 128)
        wp = np.zeros((2, 128), np.float32)
        wp[0, :64] = -NEGC
        wp[1, 64:] = -NEGC
        vmu[0:2, wsl] = wp.astype(BF16)

        ins.append(dict(qt=qt, kt=kt, ktv=ktv, vvaug=vvaug, vaug=vaug,
                        vmu=vmu, tri=tri))
    return ins


def kernel(q, k, v, sm_scale):
    from concourse.bass_utils import run_bass_kernel_spmd

    smv = float(np.asarray(sm_scale, dtype=np.float32))
    nc = _get_program(smv)
    ins = _host_inputs(q, k, v, sm_scale)
    res = run_bass_kernel_spmd(nc, ins, core_ids=list(range(H)))
    outs = []
    for h in range(H):
        o = res.results[h]["o"]  # [128, NPAIR, 129]
        o = o.transpose(1, 0, 2).reshape(S, D + 1)
        outs.append(o[:, :D] / o[:, D : D + 1])
    out = np.stack(outs, axis=0)[None]
    return out.astype(np.float32)


# revision 39
# speedup vs baseline: 1.2293x; 1.2293x over previous
"""Block-sparse attention (local + vertical-strided causal mask) on 8 TRN2 cores.

Sharding: one head per NeuronCore (H=8, n_cores=8).

Per-core device algorithm (head h, residue r = 7-h):
  The 4096x4096 score matrix is processed at 128x128 granularity:
  "pair" i = q block-rows (2i, 2i+1) (128 q tokens), "chunk" = 128 k tokens
  (2 mask blocks of 64). Local window -> chunks c in [i-8, i] of K itself;
  vertical-strided blocks -> host-gathered K_vert (6 blocks of 64, kb = 8j+r),
  processed as 3 chunks shared by all cores.

  S^T orientation: S^T[k,q] = kT_chunk.T @ qT_pair  (PE, bf16; sm_scale
  pre-folded into qT on host)
  window-start / vert-validity masks: rank-2 additive -C matmuls into the
  same PSUM region (PE); exp underflows those entries to exact 0
  P^T = exp(S^T)                                     (ACT, one call per group)
  diag triangle: multiplicative bf16 mask            (DVE)
  oacc[q, 0:129] += P^T_chunk.T @ [V | 1]_chunk      (PE, PSUM-accumulated)
  col 128 of oacc = softmax denominator; copied PSUM->SBUF (DVE) and stored
  unnormalized; the host divides by the denominator column.
"""

import numpy as np
import ml_dtypes

BF16 = ml_dtypes.bfloat16

H = 8
S = 4096
D = 128
BLK = 64
NB = S // BLK        # 64 block rows
NPAIR = NB // 2      # 32 row pairs
NVSLOT = 6           # usable vertical slots (kb = 8j + r <= 47)
NVC = NVSLOT // 2    # 3 vertical chunks
GROUP = 8            # PSUM staging slots per exp group (8 * 128 f32 = 2 banks)

NEGC = 28672.0       # additive mask constant; bf16-exact, exp() underflows to 0


def make_schedule():
    """Global ordered visit list. visit = (kind, idx, pair)
    kind "local": idx = chunk c (k blocks 2c, 2c+1)
    kind "vert":  idx = vc (K_vert slots 2vc, 2vc+1; pair i gets vc iff
    8*vc+8 <= i, i.e. the vert chunk lies fully before the local window)

    Wavefront order: all visits of pair i together (vert, then locals old
    to new, diag last) — input demand is then sequential in q and grows by
    one k-chunk per pair, so DMA staging never starves the pipeline."""
    visits = []
    for i in range(NPAIR):
        for vc in range(NVC):
            if 8 * vc + 8 <= i:
                visits.append(("vert", vc, i))
        for c in range(max(0, i - 8), i + 1):
            visits.append(("local", c, i))
    return visits


def mask_visit_order():
    """Visits that need an additive rank-2 mask, in schedule order.
    vert: per-(vc, i) validity — pruned when every head's slots are valid
    (kb_max = 16*vc+8+r <= 2i-16 for all r < 8, i.e. i >= 8*vc+16);
    local with idx == i-8: window-start mask (always needed)."""
    out = []
    for kind, idx, i in make_schedule():
        if kind == "vert" and i < 8 * idx + 16:
            out.append((kind, idx, i))
        elif kind == "local" and idx == i - 8:
            out.append((kind, idx, i))
    return out


NMASK = len(mask_visit_order())

_PROGRAMS = {}


def _build_program(sm_scale, pv_delay=4, group=GROUP, stage_bufs=2, pt_bufs=None,
                   ob_bufs=3):
    if pt_bufs is None:
        pt_bufs = pv_delay + 2
    import concourse.bass as bass
    import concourse.mybir as mybir
    import concourse.tile as tile
    from concourse import bacc

    fp32 = mybir.dt.float32
    bf16 = mybir.dt.bfloat16

    nc = bacc.Bacc("TRN2", target_bir_lowering=False, debug=False, num_devices=H)

    qt_d = nc.dram_tensor("qt", [D, S], bf16, kind="ExternalInput").ap()
    kt_d = nc.dram_tensor("kt", [D, S], bf16, kind="ExternalInput").ap()
    ktv_d = nc.dram_tensor("ktv", [D, NVSLOT * BLK], bf16, kind="ExternalInput").ap()
    vvaug_d = nc.dram_tensor("vvaug", [128, NVC * (D + 1)], bf16,
                             kind="ExternalInput").ap()
    vaug_d = nc.dram_tensor("vaug", [128, NPAIR, D + 1], bf16, kind="ExternalInput").ap()
    # vmu = per-mask-visit [128, 128] lhsT slices (rows 0/1 = invalid
    # indicators, rest zero) + wpat last; padded to K=128 so the mask
    # matmuls use the standard PE tile config (K=2 configs run ~2.7x slower)
    vmu_d = nc.dram_tensor("vmu", [128, (NMASK + 1) * 128], bf16,
                           kind="ExternalInput").ap()
    tri_d = nc.dram_tensor("tri", [128, 128], bf16, kind="ExternalInput").ap()
    o_d = nc.dram_tensor("o", [128, NPAIR, D + 1], fp32, kind="ExternalOutput").ap()

    visits = make_schedule()
    first = {}
    last = {}
    for g, (kind, idx, i) in enumerate(visits):
        first.setdefault(i, g)
        last[i] = g
    # PSUM start_tensor_calc zeroes the full 2KB bank (zero-region), so only
    # the first matmul touching an oacc tile may carry start=True.
    tile_first = {}
    for g, (kind, idx, i) in enumerate(visits):
        tile_first.setdefault(i // 3, g)
    mask_idx = {v: mi for mi, v in enumerate(mask_visit_order())}

    with tile.TileContext(nc) as tc:
        with (
            tc.tile_pool(name="big", bufs=1) as big,
            tc.tile_pool(name="stage", bufs=stage_bufs, space="PSUM") as stagep,
            tc.tile_pool(name="oacc", bufs=4, space="PSUM") as oaccp,
            tc.tile_pool(name="pt", bufs=pt_bufs) as ptp,
            tc.tile_pool(name="ob", bufs=ob_bufs) as obp,
        ):
            _emit_body(nc, tc, locals(), sm_scale, pv_delay=pv_delay, group=group)
    nc.compile()
    return nc


def _emit_body(nc, tc, env, sm_scale, pv_delay=4, group=GROUP):
    GROUP = group
    import concourse.mybir as mybir

    fp32 = mybir.dt.float32
    bf16 = mybir.dt.bfloat16
    big, stagep, oaccp, ptp, obp = (
        env["big"], env["stagep"], env["oaccp"], env["ptp"], env["obp"]
    )
    qt_d, kt_d, ktv_d, vvaug_d, vaug_d, vmu_d, tri_d, o_d = (
        env["qt_d"], env["kt_d"], env["ktv_d"], env["vvaug_d"], env["vaug_d"],
        env["vmu_d"], env["tri_d"], env["o_d"],
    )
    visits, first, last, tile_first, mask_idx = (
        env["visits"], env["first"], env["last"], env["tile_first"],
        env["mask_idx"],
    )
    n_groups = (len(visits) + GROUP - 1) // GROUP

    qt = big.tile([D, S], bf16)
    kt = big.tile([D, S], bf16)
    ktv = big.tile([D, NVSLOT * BLK], bf16)
    vvaug = big.tile([128, NVC * (D + 1)], bf16)
    vaug = big.tile([128, NPAIR, D + 1], bf16)
    vmu = big.tile([128, (NMASK + 1) * 128], bf16)
    tri = big.tile([128, 128], bf16)

    wpat = vmu[:, 0:128]

    # Inputs spread across the three DMA queues (SP-HW, ACT-HW, Pool-SW) so
    # the first groups' data lands fast; later chunks trail in use order.
    def kt_dma(a, b):
        nc.sync.dma_start(out=kt[:, a:b], in_=kt_d[:, a:b])

    def qt_dma(a, b):
        nc.scalar.dma_start(out=qt[:, a:b], in_=qt_d[:, a:b])

    def vaug_dma(a, b):
        nc.gpsimd.dma_start(out=vaug[:, a:b], in_=vaug_d[:, a:b])

    wtile = big.tile([128, 512], bf16)
    nc.vector.memset(wtile[:], 0.0)

    def vmu_dma(a, b, eng):
        eng.dma_start(out=vmu[:, a:b], in_=vmu_d[:, a:b])

    kt_dma(0, 512)
    kt_dma(512, 1024)
    nc.sync.dma_start(out=ktv[:], in_=ktv_d[:])
    kt_dma(1024, 2048)
    kt_dma(2048, 4096)
    vmu_dma(1152, 3200, nc.sync)
    vmu_dma(3200, (NMASK + 1) * 128, nc.sync)
    qt_dma(0, 1024)
    nc.scalar.dma_start(out=vvaug[:], in_=vvaug_d[:])
    qt_dma(1024, 2048)
    qt_dma(2048, 4096)
    # vmu staged in mask-emission use order; wpat is slice 0
    nc.gpsimd.dma_start(out=tri[:], in_=tri_d[:])
    vmu_dma(0, 640, nc.gpsimd)
    vaug_dma(0, 6)
    vmu_dma(640, 1152, nc.gpsimd)
    vaug_dma(6, 18)
    vaug_dma(18, 32)

    # PE p-state warmup: stream throwaway matmuls on a memset tile so the
    # array is ramping while the first inputs arrive.
    warm = stagep.tile([128, GROUP * 128], fp32, tag="stage")
    for w in range(6):
        nc.tensor.matmul(
            warm[:, (w % 2) * 512 : (w % 2 + 1) * 512],
            wtile[:, 0:128],
            wtile[:, 0:512],
            start=True,
            stop=True,
            skip_group_check=True,
        )

    oacc_tiles = {}  # pair-group (i//3) -> psum tile [128, 3, 129]
    pending_pv = []  # software pipeline: PV of group gi-d emitted
    # after S^T of group gi so PE streams while ACT/DVE process gi-d

    for gi in range(n_groups):
        gvis = visits[gi * GROUP : (gi + 1) * GROUP]
        n = len(gvis)
        stage = stagep.tile([128, GROUP * 128], fp32, tag="stage")
        ptt = ptp.tile([128, GROUP * 128], bf16, tag="pt")

        # --- S^T matmuls, batched over runs of consecutive pairs
        # sharing one k-chunk, split at 4-slot (one PSUM bank) bounds.
        # start=True only on the first run per bank (bank zero-region).
        s = 0
        seen_banks = set()
        while s < n:
            kind, idx, i0 = gvis[s]
            e = s + 1
            while (
                e < n
                and e % 4 != 0
                and gvis[e][0] == kind
                and gvis[e][1] == idx
                and gvis[e][2] == gvis[e - 1][2] + 1
            ):
                e += 1
            ln = e - s
            lhsT = (
                kt[:, idx * 128 : (idx + 1) * 128]
                if kind == "local"
                else ktv[:, idx * 128 : (idx + 1) * 128]
            )
            bank = s // 4
            nc.tensor.matmul(
                stage[:, s * 128 : e * 128],
                lhsT,
                qt[:, i0 * 128 : (i0 + ln) * 128],
                start=bank not in seen_banks,
                stop=True,
                skip_group_check=True,
            )
            seen_banks.add(bank)
            # rank-2 additive masks (window-start / vert validity) for the
            # slots of this run, accumulated into the same PSUM region
            for s2 in range(s, e):
                mv = mask_idx.get(gvis[s2])
                if mv is not None:
                    nc.tensor.matmul(
                        stage[:, s2 * 128 : (s2 + 1) * 128],
                        vmu[:, (mv + 1) * 128 : (mv + 2) * 128],
                        wpat,
                        start=False,
                        stop=True,
                        skip_group_check=True,
                    )
            s = e

        if len(pending_pv) >= pv_delay:
            pending_pv.pop(0)()

        # --- exp for the group (sm_scale pre-folded into qt on host)
        nc.scalar.activation(
            out=ptt[:, 0 : n * 128],
            in_=stage[:, 0 : n * 128],
            func=mybir.ActivationFunctionType.Exp,
        )

        # --- diag triangle mask (DVE, multiplicative bf16)
        for s, (kind, idx, i) in enumerate(gvis):
            if kind == "local" and idx == i:
                sl = slice(s * 128, (s + 1) * 128)
                nc.vector.tensor_mul(ptt[:, sl], ptt[:, sl], tri[:])

        # --- PV matmuls + epilogue (deferred pv_delay groups)
        def make_pv(gi, gvis, ptt):
            def emit_pv():
                for s, (kind, idx, i) in enumerate(gvis):
                    g = gi * GROUP + s
                    pg = i // 3
                    if pg not in oacc_tiles:
                        oacc_tiles[pg] = oaccp.tile(
                            [128, 3, D + 1], fp32, tag="oacc", name=f"oacc{pg}"
                        )
                    oacc = oacc_tiles[pg]
                    if kind == "local":
                        rhs = vaug[:, idx]
                    else:
                        rhs = vvaug[:, idx * (D + 1) : (idx + 1) * (D + 1)]
                    nc.tensor.matmul(
                        oacc[:, i % 3],
                        ptt[:, s * 128 : (s + 1) * 128],
                        rhs,
                        start=(g == tile_first[i // 3]),
                        stop=(g == last[i]),
                        skip_group_check=True,
                    )
                    # epilogue once per oacc tile (after its last pair
                    # closes): one DVE read of the PSUM bank into SBUF,
                    # then an unnormalized store (host divides by col 128).
                    pg_pairs = [p for p in (3 * pg, 3 * pg + 1, 3 * pg + 2)
                                if p < NPAIR]
                    if i == pg_pairs[-1] and g == last[i]:
                        npp = len(pg_pairs)
                        osb = obp.tile([128, 3, D + 1], fp32, tag="osb")
                        nc.vector.tensor_copy(osb[:, 0:npp], oacc[:, 0:npp])
                        nc.sync.dma_start(
                            out=o_d[:, 3 * pg : 3 * pg + npp, :],
                            in_=osb[:, 0:npp],
                        )
            return emit_pv

        pending_pv.append(make_pv(gi, gvis, ptt))
    for f in pending_pv:
        f()


def _get_program(smv=0.08838834764831845):
    key = float(smv)
    if key not in _PROGRAMS:
        _PROGRAMS[key] = _build_program(key)
    return _PROGRAMS[key]


def _host_inputs(q, k, v, sm_scale):
    """Per-core input dicts (host-side shard + layout)."""
    q = np.asarray(q, dtype=np.float32)
    k = np.asarray(k, dtype=np.float32)
    v = np.asarray(v, dtype=np.float32)
    smv = float(np.asarray(sm_scale, dtype=np.float32))

    p = np.arange(128)
    tri = np.zeros((128, 128), dtype=BF16)
    tri[p[:, None] <= p[None, :]] = BF16(1.0)

    morder = mask_visit_order()
    ins = []
    for h in range(H):
        r = 7 - h
        qh, kh, vh = q[0, h], k[0, h], v[0, h]
        qt = np.ascontiguousarray((qh * smv).T).astype(BF16)
        kt = np.ascontiguousarray(kh.T).astype(BF16)
        vblocks = [8 * j + r for j in range(NVSLOT)]
        kv = np.concatenate([kh[b * BLK : (b + 1) * BLK] for b in vblocks], axis=0)
        ktv = np.ascontiguousarray(kv.T).astype(BF16)  # [128, 384]
        vaug = np.concatenate(
            [vh, np.ones((S, 1), np.float32)], axis=1
        ).astype(BF16)  # [4096, 129]
        vaug = np.ascontiguousarray(
            vaug.reshape(NPAIR, 128, D + 1).transpose(1, 0, 2)
        )  # [128, 32, 129]
        vv = np.concatenate([vh[b * BLK : (b + 1) * BLK] for b in vblocks], axis=0)
        vvaug = np.concatenate([vv, np.ones((NVSLOT * BLK, 1), np.float32)], axis=1)
        vvaug = np.ascontiguousarray(
            vvaug.astype(BF16).reshape(NVC, 128, D + 1).transpose(1, 0, 2)
        ).reshape(128, NVC * (D + 1))  # [128, 387]

        # vmu: per-mask-visit invalid indicators in rows 0 (cols<64) and
        # 1 (cols>=64), rows 2..127 zero; wpat appended last.
        vmu = np.zeros((128, (NMASK + 1) * 128), dtype=BF16)
        for mi, (kind, idx, i) in enumerate(morder):
            sl = slice((mi + 1) * 128, (mi + 2) * 128)
            if kind == "vert":
                slot = 2 * idx + (p >= 64).astype(np.int64)
                kb = 8 * slot + r
                u0 = (kb > 2 * i - 16).astype(np.float32)       # invalid for qb=2i
                u1 = (kb > 2 * i + 1 - 16).astype(np.float32)   # invalid for qb=2i+1
            else:  # window-start: valid iff (p >= 64 and col < 64)
                u0 = (p < 64).astype(np.float32)
                u1 = np.ones(128, np.float32)
            vmu[0, sl] = u0.astype(BF16)
            vmu[1, sl] = u1.astype(BF16)
        wsl = slice(0, 128)
        wp = np.zeros((2, 128), np.float32)
        wp[0, :64] = -NEGC
        wp[1, 64:] = -NEGC
        vmu[0:2, wsl] = wp.astype(BF16)

        ins.append(dict(qt=qt, kt=kt, ktv=ktv, vvaug=vvaug, vaug=vaug,
                        vmu=vmu, tri=tri))
    return ins


def kernel(q, k, v, sm_scale):
    from concourse.bass_utils import run_bass_kernel_spmd

    smv = float(np.asarray(sm_scale, dtype=np.float32))
    nc = _get_program(smv)
    ins = _host_inputs(q, k, v, sm_scale)
    res = run_bass_kernel_spmd(nc, ins, core_ids=list(range(H)))
    outs = []
    for h in range(H):
        o = res.results[h]["o"]  # [128, NPAIR, 129]
        o = o.transpose(1, 0, 2).reshape(S, D + 1)
        outs.append(o[:, :D] / o[:, D : D + 1])
    out = np.stack(outs, axis=0)[None]
    return out.astype(np.float32)
